# revision 1
# baseline (speedup 1.0000x reference)
"""Trainium2 Bass kernel for nn_MaskedSelfAttention (causal, QK rms-norm).

Sharding: 8 cores = 2 (batch) x 4 (head groups of 4 heads).
Each core computes qkv projection for its heads, causal flash-style
attention (no max subtraction -- scores are bounded by rms norm), and a
partial FC output over its heads' feature slice. Host sums the 4 partials
per batch.

Self-contained: hardcodes shapes from the problem spec.
"""

import numpy as np

import concourse.bacc as bacc
import concourse.mybir as mybir
import concourse.tile as tile
from concourse.bass_utils import run_bass_kernel_spmd

B, L, D = 2, 2048, 1024
DH = 64
NH = D // DH            # 16 heads total
P = 128
NHC = 4                 # heads per core
E3 = 3 * NHC * DH       # 768 qkv rows per core
LB = L // P             # 16 l-blocks
KB = D // P             # 8 contraction blocks
EPS = 1e-5
F32 = mybir.dt.float32
F32R = mybir.dt.float32r


def R(ap):
    return ap.bitcast(F32R)

FX = mybir.ActivationFunctionType
MULT = mybir.AluOpType.mult

_CACHE = {}


def _build_nc():
    nc = bacc.Bacc("TRN2", target_bir_lowering=False, debug=False)

    xT = nc.dram_tensor("xT", (D, L), F32, kind="ExternalInput").ap()
    wqkvT = nc.dram_tensor("wqkvT", (D, E3), F32, kind="ExternalInput").ap()
    wfcT = nc.dram_tensor("wfcT", (NHC * DH, D), F32, kind="ExternalInput").ap()
    triu = nc.dram_tensor("triu", (P, P), F32, kind="ExternalInput").ap()
    wqk = nc.dram_tensor("wqk", (P, 1), F32, kind="ExternalInput").ap()
    ident = nc.dram_tensor("ident", (P, P), F32, kind="ExternalInput").ap()
    sel = nc.dram_tensor("sel", (NHC, NHC * DH), F32, kind="ExternalInput").ap()
    outp = nc.dram_tensor("outp", (L, D), F32, kind="ExternalOutput").ap()

    with tile.TileContext(nc) as tc:
        with (
            tc.tile_pool(name="cpool", bufs=1) as cpool,
            tc.tile_pool(name="wpool", bufs=1) as wpool,
            tc.tile_pool(name="ppool", bufs=1) as ppool,
            tc.tile_pool(name="xpool", bufs=3) as xpool,
            tc.tile_pool(name="work", bufs=8) as work,
            tc.tile_pool(name="ptpool", bufs=6) as ptpool,
            tc.tile_pool(name="opool", bufs=3) as opool,
        ):
            ident_sb = cpool.tile([P, P], F32)
            nc.sync.dma_start(ident_sb, ident)
            identr_sb = cpool.tile([P, P], F32R)
            nc.sync.dma_start(identr_sb, R(ident))
            triu_sb = cpool.tile([P, P], F32)
            nc.sync.dma_start(triu_sb, triu)
            wqk_sb = cpool.tile([P, 1], F32)
            nc.sync.dma_start(wqk_sb, wqk)
            sel_sb = cpool.tile([NHC, NHC * DH], F32R)
            nc.sync.dma_start(sel_sb, R(sel))
            biasq = cpool.tile([P, 1], F32)
            nc.vector.memset(biasq, DH * EPS)

            wqkv_sb = wpool.tile([P, KB, E3], F32R)
            wqkvT_r = wqkvT.rearrange("(ko p) e -> p ko e", p=P)
            nc.sync.dma_start(wqkv_sb[:, 0:1, :], R(wqkvT_r[:, 0:1, :]))
            nc.sync.dma_start(wqkv_sb[:, 1:KB, :], R(wqkvT_r[:, 1:KB, :]))
            wfc_sb = wpool.tile([P, 2, D], F32R)
            nc.sync.dma_start(wfc_sb, R(wfcT.rearrange("(g p) e -> p g e", p=P)))

            # persistent activations (per-partition fp32 bytes in comments)
            qT = ppool.tile([P, 2, L], F32R)      # 16KB  [dh-pair, hp, l]
            kT = ppool.tile([P, 2, L], F32R)      # 16KB
            vext = ppool.tile([P, LB, NHC, DH + 1], F32R)  # 16.25KB, col DH = ones
            oT = ppool.tile([P, 2, L], F32R)      # 16KB  unnorm O^T, normed in place
            rec = ppool.tile([NHC, L], F32R)      # 1/denom, free-major
            dnT = ppool.tile([P, LB, NHC], F32)    # denom, lq-partition-major
            recT = ppool.tile([P, LB, NHC], F32)

            onesf = cpool.tile([P, 1], F32)
            nc.vector.memset(onesf, 1.0)
            nc.vector.tensor_copy(
                vext[:, :, :, DH : DH + 1],
                onesf[:, :, None, None].to_broadcast((P, LB, NHC, 1)),
            )

            # ---- Phase A: qkv projection (l,e') + rms norm + transpose q,k ----
            with (
                tc.tile_pool(name="psA", bufs=2, space="PSUM") as psA,
                tc.tile_pool(name="psT", bufs=3, space="PSUM") as psT,
            ):
                for m in range(LB):
                    # whole xT column-block for this m in one DMA: [128, 8, 128]
                    xc = xpool.tile([P, KB, P], F32R, tag="xc", name=f"xc_{m}")
                    nc.sync.dma_start(
                        xc, R(xT.rearrange("(ko p) l -> p ko l", p=P)[:, :, m * P : (m + 1) * P])
                    )
                    xts = [xc[:, k, :] for k in range(KB)]
                    # q+k in one 512-wide matmul chain, v in a 256-wide one
                    ps = psA.tile([P, 2 * NHC * DH], F32, tag="qkps", bufs=3, name=f"qkps_{m}")
                    psv = psA.tile([P, NHC * DH], F32, tag="vps", bufs=2, name=f"vps_{m}")
                    for k in range(KB):
                        nc.tensor.matmul(
                            ps,
                            lhsT=xts[k],
                            rhs=wqkv_sb[:, k, 0 : 2 * NHC * DH],
                            start=(k == 0),
                            stop=(k == KB - 1),
                        )
                    for k in range(KB):
                        nc.tensor.matmul(
                            psv,
                            lhsT=xts[k],
                            rhs=wqkv_sb[:, k, 2 * NHC * DH : 3 * NHC * DH],
                            start=(k == 0),
                            stop=(k == KB - 1),
                        )
                    nc.scalar.copy(
                        vext[:, m, :, 0:DH],
                        psv.rearrange("p (h d) -> p h d", d=DH),
                    )
                    sq = work.tile([P, 2 * NHC * DH], F32, tag="sq", name=f"sq_{m}")
                    nc.scalar.activation(sq, ps, FX.Square)
                    ssq = work.tile([P, 2 * NHC], F32, tag="ssq", name=f"ssq_{m}")
                    nc.vector.reduce_sum(
                        ssq,
                        sq.rearrange("p (h d) -> p h d", d=DH),
                        axis=mybir.AxisListType.X,
                    )
                    rin = work.tile([P, 2 * NHC], F32, tag="rin", name=f"rin_{m}")
                    # both q,k: 1/rin = 0.125 / sqrt(mean + eps); the extra 1/64
                    # vs the reference's 1/8 sdpa scale is undone by exp(scale=8)
                    nc.scalar.activation(rin, ssq, FX.Sqrt, bias=biasq[:, :], scale=1.0)
                    inv = work.tile([P, 2 * NHC], F32, tag="inv", name=f"inv_{m}")
                    nc.vector.reciprocal(inv, rin)
                    qn = work.tile([P, 2 * NHC * DH], F32R, tag="qn", name=f"qn_{m}")
                    nc.vector.tensor_tensor(
                        qn.rearrange("p (h d) -> p h d", d=DH),
                        ps.rearrange("p (h d) -> p h d", d=DH),
                        inv[:, :, None].to_broadcast((P, 2 * NHC, DH)),
                        MULT,
                    )
                    for g in range(4):  # blocks: 0,1 -> qT; 2,3 -> kT
                        dst = qT if g < 2 else kT
                        tp = psT.tile([P, P], F32R, tag="tp", name=f"tp_{m}_{g}")
                        nc.tensor.transpose(tp, qn[:, g * P : (g + 1) * P], identr_sb)
                        if g % 2 == 0:
                            nc.vector.tensor_copy(dst[:, g % 2, m * P : (m + 1) * P], tp)
                        else:
                            nc.scalar.copy(dst[:, g % 2, m * P : (m + 1) * P], tp)
                    # fold norm weights (q_norm_w * k_norm_w) into kT, per-partition
                    nc.vector.tensor_scalar_mul(
                        kT[:, :, m * P : (m + 1) * P], kT[:, :, m * P : (m + 1) * P], wqk_sb
                    )

            # ---- Phase B: attention. S^T = kT.T@qT, P^T = exp, O^T += V^T@P^T ----
            with (
                tc.tile_pool(name="psS", bufs=3, space="PSUM") as psS,
                tc.tile_pool(name="psO", bufs=2, space="PSUM") as psO,
            ):
                for hp in range(2):
                    for c in range(4):
                        oTps = [
                            psO.tile([DH + 1, 512], F32, tag="oT", name=f"oT_{hp}_{c}_{h2}")
                            for h2 in range(2)
                        ]
                        nj = 4 * c + 4
                        for j in range(nj):
                            off = max(0, j * P - c * 512)
                            W = 512 - off
                            st = psS.tile([P, 2, 512], F32, tag="sT", name=f"sT_{hp}_{c}_{j}")
                            for h2 in range(2):
                                nc.tensor.matmul(
                                    st[:, h2, 0:W],
                                    lhsT=kT[h2 * DH : (h2 + 1) * DH, hp, j * P : (j + 1) * P],
                                    rhs=qT[h2 * DH : (h2 + 1) * DH, hp, c * 512 + off : (c + 1) * 512],
                                    start=True,
                                    stop=True,
                                )
                            pt = ptpool.tile([P, 2, 512], F32R, tag="pt", name=f"pt_{hp}_{c}_{j}")
                            nc.scalar.activation(pt[:, :, 0:W], st[:, :, 0:W], FX.Exp, scale=8.0)
                            if j >= 4 * c:
                                nc.vector.tensor_tensor(
                                    pt[:, :, 0:P],
                                    pt[:, :, 0:P],
                                    triu_sb[:, None, :].to_broadcast((P, 2, P)),
                                    MULT,
                                )
                            for h2 in range(2):
                                nc.tensor.matmul(
                                    oTps[h2][:, off:512],
                                    lhsT=vext[:, j, 2 * hp + h2, :],
                                    rhs=pt[:, h2, 0:W],
                                    start=(j == 0),
                                    stop=(j == nj - 1),
                                    skip_group_check=True,
                                )
                        for h2 in range(2):
                            lh = 2 * hp + h2
                            # stage denom row at partition 64 (no partition shift),
                            # then PE-transpose 128-col pieces to lq-partition-major
                            dnc = work.tile([DH + 1, 512], F32, tag="dnc", name=f"dnc_{hp}_{c}_{h2}")
                            nc.vector.tensor_copy(dnc[DH : DH + 1, :], oTps[h2][DH : DH + 1, :])
                            dnps = psO.tile([P, NHC], F32, tag="oT", name=f"dnps_{hp}_{c}_{h2}")
                            for mi in range(4):
                                nc.tensor.transpose(
                                    dnps[:, mi : mi + 1],
                                    dnc[DH : DH + 1, mi * P : (mi + 1) * P],
                                    ident_sb[DH : DH + 1, DH : DH + 1],
                                )
                            nc.vector.tensor_copy(dnT[:, 4 * c : 4 * c + 4, lh], dnps)
                            if h2 == 0:
                                nc.vector.tensor_copy(
                                    oT[h2 * DH : (h2 + 1) * DH, hp, c * 512 : (c + 1) * 512],
                                    oTps[h2][0:DH, :],
                                )
                            else:
                                nc.scalar.copy(
                                    oT[h2 * DH : (h2 + 1) * DH, hp, c * 512 : (c + 1) * 512],
                                    oTps[h2][0:DH, :],
                                )

            # ---- Phase C: reciprocal of denominators + normalize O^T + FC ----
            with (
                tc.tile_pool(name="psC", bufs=1, space="PSUM") as psC,
                tc.tile_pool(name="psR", bufs=3, space="PSUM") as psR,
                tc.tile_pool(name="psF", bufs=4, space="PSUM") as psF,
            ):
                nc.vector.reciprocal(
                    recT.rearrange("p a b -> p (a b)"),
                    dnT.rearrange("p a b -> p (a b)"),
                )
                for c in range(4):
                    for mi in range(4):
                        m = 4 * c + mi
                        tp2 = psC.tile([NHC, P], F32, tag="recb", name=f"recb_{m}")
                        nc.tensor.transpose(tp2, recT[:, m, :], ident_sb)
                        nc.vector.tensor_copy(rec[:, m * P : (m + 1) * P], tp2)
                    for hp in range(2):
                        for h2 in range(2):
                            lh = 2 * hp + h2
                            rb = psR.tile([DH, 512], F32, tag="rb", name=f"rb_{lh}_{c}")
                            nc.tensor.matmul(
                                rb,
                                lhsT=sel_sb[:, lh * DH : (lh + 1) * DH],
                                rhs=rec[:, c * 512 : (c + 1) * 512],
                                start=True,
                                stop=True,
                            )
                            seg = oT[h2 * DH : (h2 + 1) * DH, hp, c * 512 : (c + 1) * 512]
                            nc.vector.tensor_tensor(seg, seg, rb, MULT)
                    for mi in range(4):
                        m = 4 * c + mi
                        for n in range(2):
                            fp = psF.tile([P, 512], F32, tag="fc", name=f"fc_{m}_{n}")
                            for g in range(2):
                                nc.tensor.matmul(
                                    fp,
                                    lhsT=oT[:, g, m * P : (m + 1) * P],
                                    rhs=wfc_sb[:, g, n * 512 : (n + 1) * 512],
                                    start=(g == 0),
                                    stop=(g == 1),
                                )
                            ot = opool.tile([P, 512], F32, tag="ot", name=f"ot_{m}_{n}")
                            nc.scalar.copy(ot, fp)
                            nc.sync.dma_start(outp[m * P : (m + 1) * P, n * 512 : (n + 1) * 512], ot)

    nc.compile()
    return nc


def _make_in_maps(x, w_qkv, w_fc, q_norm_w, k_norm_w):
    triu_f = np.triu(np.ones((P, P), dtype=np.float32))
    ident = np.eye(P, dtype=np.float32)
    sel = np.kron(np.eye(NHC), np.ones((1, DH))).astype(np.float32)
    wqk = np.tile((q_norm_w * k_norm_w).astype(np.float32), 2).reshape(P, 1)
    wqkvT = {}
    wfcTs = {}
    for hg in range(4):
        h0 = hg * NHC
        rows = np.concatenate(
            [
                w_qkv[h0 * DH : (h0 + NHC) * DH],
                w_qkv[D + h0 * DH : D + (h0 + NHC) * DH],
                w_qkv[2 * D + h0 * DH : 2 * D + (h0 + NHC) * DH],
            ],
            axis=0,
        )
        wqkvT[hg] = np.ascontiguousarray(rows.T.astype(np.float32))
        wfcTs[hg] = np.ascontiguousarray(w_fc.T[h0 * DH : (h0 + NHC) * DH].astype(np.float32))
    xTs = [np.ascontiguousarray(x[b].T.astype(np.float32)) for b in range(B)]
    in_maps = []
    for core in range(8):
        b, hg = core // 4, core % 4
        in_maps.append(
            {
                "xT": xTs[b],
                "wqkvT": wqkvT[hg],
                "wfcT": wfcTs[hg],
                "triu": triu_f,
                "wqk": wqk,
                "ident": ident,
                "sel": sel,
            }
        )
    return in_maps


def _is_causal(mask):
    idx = np.arange(mask.shape[0])
    return mask.shape == (L, L) and bool(np.all(mask == (idx[None, :] <= idx[:, None])))


def _reference_numpy(x, mask, w_qkv, w_fc, q_norm_w, k_norm_w, subset_attention_size):
    # slow but general fallback (only used if mask is not causal)
    b, l, d = x.shape
    qkv = x @ w_qkv.T
    q, k, v = np.split(qkv, 3, axis=-1)

    def heads(t):
        return t.reshape(b, l, NH, DH).transpose(0, 2, 1, 3)

    def rms(t, w):
        return t * (1.0 / np.sqrt(np.mean(t * t, -1, keepdims=True) + EPS)) * w

    q, k, v = heads(q), heads(k), heads(v)
    q, k = rms(q, q_norm_w), rms(k, k_norm_w)

    def sdpa(q, k, v, m):
        s = np.einsum("bhqd,bhkd->bhqk", q, k) / np.sqrt(DH)
        s = np.where(m[None, None], s, -1e30)
        s = s - s.max(-1, keepdims=True)
        p = np.exp(s)
        p /= p.sum(-1, keepdims=True)
        return np.einsum("bhqk,bhkd->bhqd", p, v)

    S = int(subset_attention_size) if subset_attention_size is not None else None
    if S is not None and S < l:
        o = np.concatenate(
            [
                sdpa(q[:, :, :S], k[:, :, :S], v[:, :, :S], mask[:S, :S]),
                sdpa(q[:, :, S:], k, v, mask[S:, :]),
            ],
            axis=2,
        )
    else:
        o = sdpa(q, k, v, mask)
    o = o.transpose(0, 2, 1, 3).reshape(b, l, d)
    return (o @ w_fc.T).astype(np.float32)


def kernel(**inputs):
    x = np.asarray(inputs["x"], dtype=np.float32)
    mask = np.asarray(inputs["mask"])
    w_qkv = np.asarray(inputs["w_qkv"], dtype=np.float32)
    w_fc = np.asarray(inputs["w_fc"], dtype=np.float32)
    q_norm_w = np.asarray(inputs["q_norm_w"], dtype=np.float32)
    k_norm_w = np.asarray(inputs["k_norm_w"], dtype=np.float32)

    if not _is_causal(mask):
        return _reference_numpy(
            x, mask, w_qkv, w_fc, q_norm_w, k_norm_w, inputs.get("subset_attention_size")
        )

    if "nc" not in _CACHE:
        _CACHE["nc"] = _build_nc()
    nc = _CACHE["nc"]

    in_maps = _make_in_maps(x, w_qkv, w_fc, q_norm_w, k_norm_w)
    res = run_bass_kernel_spmd(nc, in_maps, core_ids=list(range(8)))
    parts = [res.results[i]["outp"] for i in range(8)]
    out = np.empty((B, L, D), dtype=np.float32)
    for b in range(B):
        acc = np.zeros((L, D), dtype=np.float64)
        for hg in range(4):
            acc += parts[b * 4 + hg]
        out[b] = acc.astype(np.float32)
    return out



# revision 16
# speedup vs baseline: 1.0393x; 1.0393x over previous
"""Trainium2 Bass kernel for nn_MaskedSelfAttention (causal, QK rms-norm).

Sharding: 8 cores = 2 (batch) x 4 (head groups of 4 heads).
Each core computes qkv projection for its heads, causal attention
(no max subtraction -- scores are bounded by rms norm), and a partial FC
output over its heads' feature slice. Host sums the 4 partials per batch.

v2: bf16 matmuls + storage (fp32 PSUM accumulation, fp32 denominators),
software-pipelined phases so the PE never waits on ACT/DVE, exp split
between ACT (exact) and DVE (int16 Schraudolph bit-trick) to unbottleneck
the scalar engine, copies spread across ACT/DVE/GpSimd, bf16 output.

Self-contained: hardcodes shapes from the problem spec.
"""

import numpy as np
import ml_dtypes

import concourse.bacc as bacc
import concourse.mybir as mybir
import concourse.tile as tile
from concourse.bass_utils import run_bass_kernel_spmd

B, L, D = 2, 2048, 1024
DH = 64
NH = D // DH            # 16 heads total
P = 128
NHC = 4                 # heads per core
E3 = 3 * NHC * DH       # 768 qkv rows per core
LB = L // P             # 16 l-blocks
KB = D // P             # 8 contraction blocks
EPS = 1e-5
F32 = mybir.dt.float32
F32R = mybir.dt.float32r
BF16 = mybir.dt.bfloat16
I16 = mybir.dt.int16

# Schraudolph exp for bf16: exp(x) ~= bitcast_bf16(int16(x*A + Bc))
SCH_A = 128.0 / float(np.log(2.0))
SCH_B = 16256.0 - 4.5

BF = ml_dtypes.bfloat16


def R(ap):
    return ap.bitcast(F32R)


FX = mybir.ActivationFunctionType
MULT = mybir.AluOpType.mult
ADD = mybir.AluOpType.add

_CACHE = {}


def _build_nc():
    nc = bacc.Bacc("TRN2", target_bir_lowering=False, debug=False)

    xT = nc.dram_tensor("xT", (D, L), BF16, kind="ExternalInput").ap()
    wqkvT = nc.dram_tensor("wqkvT", (D, E3), BF16, kind="ExternalInput").ap()
    wfcT = nc.dram_tensor("wfcT", (NHC * DH, D), BF16, kind="ExternalInput").ap()
    triu = nc.dram_tensor("triu", (P, P), BF16, kind="ExternalInput").ap()
    wqk = nc.dram_tensor("wqk", (P, 1), F32, kind="ExternalInput").ap()
    identb = nc.dram_tensor("identb", (P, P), BF16, kind="ExternalInput").ap()
    identf = nc.dram_tensor("identf", (P, P), F32, kind="ExternalInput").ap()
    selhp = nc.dram_tensor("selhp", (NHC, 2, P), F32, kind="ExternalInput").ap()
    outp = nc.dram_tensor("outp", (L, D), BF16, kind="ExternalOutput").ap()

    with tile.TileContext(nc) as tc:
        with (
            tc.tile_pool(name="cpool", bufs=1) as cpool,
            tc.tile_pool(name="wpool", bufs=1) as wpool,
            tc.tile_pool(name="ppool", bufs=1) as ppool,
            tc.tile_pool(name="xpool", bufs=3) as xpool,
            tc.tile_pool(name="work", bufs=8) as work,
            tc.tile_pool(name="qnpool", bufs=3) as qnpool,
            tc.tile_pool(name="ptpool", bufs=4) as ptpool,
            tc.tile_pool(name="dnpool", bufs=2) as dnpool,
            tc.tile_pool(name="opool", bufs=4) as opool,
        ):
            # ---- constants / weights ----
            identb_sb = cpool.tile([P, P], BF16)
            nc.sync.dma_start(identb_sb, identb)
            identf_sb = cpool.tile([P, P], F32)
            nc.sync.dma_start(identf_sb, identf)
            triu_sb = cpool.tile([P, P], BF16)
            nc.sync.dma_start(triu_sb, triu)
            wqk_sb = cpool.tile([P, 1], F32)
            nc.sync.dma_start(wqk_sb, wqk)
            selhp_sb = cpool.tile([NHC, 2, P], F32R)
            nc.sync.dma_start(selhp_sb, R(selhp))
            biasq = cpool.tile([P, 1], F32)
            nc.vector.memset(biasq, DH * EPS)

            wqkv_sb = wpool.tile([P, KB, E3], BF16)
            wqkvT_r = wqkvT.rearrange("(ko p) e -> p ko e", p=P)
            nc.sync.dma_start(wqkv_sb[:, 0:1, :], wqkvT_r[:, 0:1, :])
            nc.sync.dma_start(wqkv_sb[:, 1:KB, :], wqkvT_r[:, 1:KB, :])
            wfc_sb = wpool.tile([P, 2, D], BF16)
            nc.sync.dma_start(wfc_sb, wfcT.rearrange("(g p) e -> p g e", p=P))

            # persistent activations (per-partition bytes in comments)
            qT = ppool.tile([P, 2, L], BF16)              # 8KB [dh-pair, hp, l]
            kT = ppool.tile([P, 2, L], BF16)              # 8KB
            vext = ppool.tile([P, LB, NHC, DH + 1], BF16)  # 8.1KB, col DH = ones
            oText = ppool.tile([P, 2, L], BF16)           # 8KB O^T, normed in place
            dnT = ppool.tile([P, LB, NHC], F32)           # denom, lq-partition-major
            recT = ppool.tile([P, LB, NHC], F32)          # 1/denom
            rec = ppool.tile([NHC, L], F32R)              # 1/denom, head-major

            onesb = cpool.tile([P, 1], BF16)
            nc.vector.memset(onesb, 1.0)
            nc.vector.tensor_copy(
                vext[:, :, :, DH : DH + 1],
                onesb[:, :, None, None].to_broadcast((P, LB, NHC, 1)),
            )

            # ---- Phase A: qkv projection (l,e') + rms norm + transpose q,k ----
            xT_r = xT.rearrange("(ko p) l -> p ko l", p=P)
            qn_tiles = [None] * LB

            def emit_transposes(m, psT):
                qn = qn_tiles[m]
                for g in range(4):  # blocks: 0,1 -> qT; 2,3 -> kT
                    tp = psT.tile([P, P], BF16, tag="tp", name=f"tp_{m}_{g}")
                    nc.tensor.transpose(tp, qn[:, g * P : (g + 1) * P], identb_sb)
                    if g < 2:
                        nc.vector.tensor_copy(qT[:, g, m * P : (m + 1) * P], tp)
                    else:
                        # fold q_norm_w*k_norm_w into kT during the copy
                        nc.scalar.activation(
                            kT[:, g % 2, m * P : (m + 1) * P], tp, FX.Copy,
                            scale=wqk_sb[:, :],
                        )

            with (
                tc.tile_pool(name="psA", bufs=2, space="PSUM") as psA,
                tc.tile_pool(name="psT", bufs=3, space="PSUM") as psT,
            ):
                for m in range(LB):
                    xc = xpool.tile([P, KB, P], BF16, tag="xc", name=f"xc_{m}")
                    nc.sync.dma_start(xc, xT_r[:, :, m * P : (m + 1) * P])
                    ps = psA.tile([P, 2 * NHC * DH], F32, tag="qkps", bufs=3,
                                  name=f"qkps_{m}")
                    psv = psA.tile([P, NHC * DH], F32, tag="vps", bufs=2,
                                   name=f"vps_{m}")
                    for k in range(KB):
                        nc.tensor.matmul(
                            ps, lhsT=xc[:, k, :],
                            rhs=wqkv_sb[:, k, 0 : 2 * NHC * DH],
                            start=(k == 0), stop=(k == KB - 1),
                        )
                    for k in range(KB):
                        nc.tensor.matmul(
                            psv, lhsT=xc[:, k, :],
                            rhs=wqkv_sb[:, k, 2 * NHC * DH : 3 * NHC * DH],
                            start=(k == 0), stop=(k == KB - 1),
                        )
                    # transposes of the previous iteration keep PE busy while
                    # this iteration's norm chain runs on ACT/DVE
                    if m > 0:
                        emit_transposes(m - 1, psT)
                    nc.scalar.copy(
                        vext[:, m, :, 0:DH],
                        psv.rearrange("p (h d) -> p h d", d=DH),
                    )
                    sq = work.tile([P, 2 * NHC * DH], F32, tag="sq", name=f"sq_{m}")
                    nc.scalar.activation(sq, ps, FX.Square)
                    ssq = work.tile([P, 2 * NHC], F32, tag="ssq", name=f"ssq_{m}")
                    nc.vector.reduce_sum(
                        ssq, sq.rearrange("p (h d) -> p h d", d=DH),
                        axis=mybir.AxisListType.X,
                    )
                    rin = work.tile([P, 2 * NHC], F32, tag="rin", name=f"rin_{m}")
                    # both q,k: 1/rin = 0.125 / sqrt(mean + eps); the extra 1/64
                    # vs the reference's 1/8 sdpa scale is undone by exp(scale=8)
                    nc.scalar.activation(rin, ssq, FX.Sqrt, bias=biasq[:, :], scale=1.0)
                    inv = work.tile([P, 2 * NHC], F32, tag="inv", name=f"inv_{m}")
                    nc.vector.reciprocal(inv, rin)
                    qn = qnpool.tile([P, 2 * NHC * DH], BF16, tag="qn", name=f"qn_{m}")
                    nc.vector.tensor_tensor(
                        qn.rearrange("p (h d) -> p h d", d=DH),
                        ps.rearrange("p (h d) -> p h d", d=DH),
                        inv[:, :, None].to_broadcast((P, 2 * NHC, DH)),
                        MULT,
                    )
                    qn_tiles[m] = qn
                emit_transposes(LB - 1, psT)

            # ---- Phase B: attention. S^T = kT.T@qT, P^T = exp, O^T += V^T@P^T ----
            ndve = [0]
            nexp = [0]
            with (
                tc.tile_pool(name="psS", bufs=2, space="PSUM") as psS,
                tc.tile_pool(name="psO", bufs=2, space="PSUM") as psO,
            ):
                for hp in range(2):
                    for c in range(4):
                        oTps = psO.tile([DH + 1, 2, 512], F32, tag="oT",
                                        name=f"oT_{hp}_{c}")
                        nj = 4 * c + 4
                        sts = [None] * nj
                        pts = [None] * nj

                        def emit_S(j, hp=hp, c=c, nj=nj, sts=sts, pts=pts):
                            off = max(0, j * P - c * 512)
                            W = 512 - off
                            st = psS.tile([P, 2, 512], F32, tag="sT",
                                          name=f"sT_{hp}_{c}_{j}")
                            for h2 in range(2):
                                nc.tensor.matmul(
                                    st[:, h2, 0:W],
                                    lhsT=kT[h2 * DH : (h2 + 1) * DH, hp,
                                            j * P : (j + 1) * P],
                                    rhs=qT[h2 * DH : (h2 + 1) * DH, hp,
                                           c * 512 + off : (c + 1) * 512],
                                    start=True, stop=True,
                                )
                            sts[j] = st

                        def emit_exp(j, hp=hp, c=c, nj=nj, sts=sts, pts=pts):
                            off = max(0, j * P - c * 512)
                            W = 512 - off
                            st = sts[j]
                            diag = j >= 4 * c
                            nexp[0] += 1
                            use_dve = (not diag) and (nexp[0] % 4 == 0)
                            if use_dve:
                                ndve[0] += 1
                                pti = ptpool.tile([P, 2, 512], I16, tag="pt",
                                                  name=f"pti_{hp}_{c}_{j}")
                                nc.vector.tensor_scalar(
                                    pti, st, 8.0 * SCH_A, SCH_B, MULT, ADD,
                                )
                                pt = pti.bitcast(BF16)
                            else:
                                pt = ptpool.tile([P, 2, 512], BF16, tag="pt",
                                                 name=f"pt_{hp}_{c}_{j}")
                                nc.scalar.activation(
                                    pt[:, :, 0:W], st[:, :, 0:W], FX.Exp, scale=8.0
                                )
                            if diag:
                                nc.vector.tensor_tensor(
                                    pt[:, :, 0:P], pt[:, :, 0:P],
                                    triu_sb[:, None, :].to_broadcast((P, 2, P)),
                                    MULT,
                                )
                            pts[j] = pt

                        def emit_PV(j, hp=hp, c=c, nj=nj, sts=sts, pts=pts,
                                    oTps=oTps):
                            off = max(0, j * P - c * 512)
                            W = 512 - off
                            pt = pts[j]
                            for h2 in range(2):
                                nc.tensor.matmul(
                                    oTps[:, h2, off:512],
                                    lhsT=vext[:, j, 2 * hp + h2, :],
                                    rhs=pt[:, h2, 0:W],
                                    start=(j == 0), stop=(j == nj - 1),
                                    skip_group_check=True,
                                )

                        emit_S(0)
                        for j in range(nj):
                            emit_exp(j)
                            if j + 1 < nj:
                                emit_S(j + 1)
                            emit_PV(j)

                        # denominators (fp32): row 64 of oTps -> dnT via PE
                        dn = dnpool.tile([1, 2, 512], F32, tag="dn",
                                         name=f"dn_{hp}_{c}")
                        nc.vector.tensor_copy(dn, oTps[DH : DH + 1, :, :])
                        dnps = psS.tile([P, 2, 512], F32, tag="sT",
                                        name=f"dnps_{hp}_{c}")
                        for h2 in range(2):
                            for mi in range(4):
                                nc.tensor.transpose(
                                    dnps[:, h2, mi : mi + 1],
                                    dn[:, h2, mi * P : (mi + 1) * P],
                                    identf_sb[0:1, 0:1],
                                )
                        nc.vector.tensor_copy(
                            dnT[:, 4 * c : 4 * c + 4, 2 * hp : 2 * hp + 2]
                            .rearrange("p a b -> p b a"),
                            dnps[:, :, 0:NHC],
                        )
                        # unnormalized O^T -> SBUF bf16
                        nc.vector.tensor_copy(
                            oText[0:DH, hp, c * 512 : (c + 1) * 512],
                            oTps[0:DH, 0, :],
                        )
                        nc.scalar.copy(
                            oText[DH : 2 * DH, hp, c * 512 : (c + 1) * 512],
                            oTps[0:DH, 1, :],
                        )

            # ---- Phase C: normalize O^T by 1/denom + FC ----
            with (
                tc.tile_pool(name="psC", bufs=2, space="PSUM") as psC,
                tc.tile_pool(name="psR", bufs=2, space="PSUM") as psR,
                tc.tile_pool(name="psF", bufs=4, space="PSUM") as psF,
            ):
                nc.vector.reciprocal(
                    recT.rearrange("p a b -> p (a b)"),
                    dnT.rearrange("p a b -> p (a b)"),
                )

                def emit_fc(m):
                    for n in range(2):
                        fp = psF.tile([P, 512], F32, tag="fc", name=f"fc_{m}_{n}")
                        for g in range(2):
                            nc.tensor.matmul(
                                fp,
                                lhsT=oText[:, g, m * P : (m + 1) * P],
                                rhs=wfc_sb[:, g, n * 512 : (n + 1) * 512],
                                start=(g == 0), stop=(g == 1),
                            )
                        ot = opool.tile([P, 512], BF16, tag="ot", name=f"ot_{m}_{n}")
                        if (2 * m + n) % 2 == 0:
                            nc.scalar.copy(ot, fp)
                        else:
                            nc.vector.tensor_copy(ot, fp)
                        nc.sync.dma_start(
                            outp[m * P : (m + 1) * P, n * 512 : (n + 1) * 512], ot
                        )

                for c in range(4):
                    for mi in range(4):
                        m = 4 * c + mi
                        tpr = psC.tile([NHC, P], F32, tag="recb", name=f"recb_{m}")
                        nc.tensor.transpose(tpr, recT[:, m, :], identf_sb)
                        if mi % 2 == 0:
                            nc.vector.tensor_copy(rec[:, m * P : (m + 1) * P], tpr)
                        else:
                            nc.scalar.copy(rec[:, m * P : (m + 1) * P], tpr)
                    for hp in range(2):
                        rb = psR.tile([P, 512], F32, tag="rb", name=f"rb_{hp}_{c}")
                        nc.tensor.matmul(
                            rb,
                            lhsT=selhp_sb[:, hp, :],
                            rhs=rec[:, c * 512 : (c + 1) * 512],
                            start=True, stop=True,
                        )
                        seg = oText[:, hp, c * 512 : (c + 1) * 512]
                        nc.vector.tensor_tensor(seg, seg, rb, MULT)
                    # FC for the previous c-group overlaps this group's
                    # normalization chain
                    if c > 0:
                        for mi in range(4):
                            emit_fc(4 * (c - 1) + mi)
                for mi in range(4):
                    emit_fc(12 + mi)

    nc.compile()
    return nc


def _make_in_maps(x, w_qkv, w_fc, q_norm_w, k_norm_w):
    triu_f = np.triu(np.ones((P, P))).astype(BF)
    identb = np.eye(P).astype(BF)
    identf = np.eye(P, dtype=np.float32)
    # selhp[h, hp, p] = 1 iff head h == 2*hp + p//DH (rb = rec broadcast over dh)
    selhp = np.zeros((NHC, 2, P), dtype=np.float32)
    for hp in range(2):
        for p in range(P):
            selhp[2 * hp + p // DH, hp, p] = 1.0
    wqk = np.tile((q_norm_w * k_norm_w).astype(np.float32), 2).reshape(P, 1)
    wqkvT = {}
    wfcTs = {}
    for hg in range(4):
        h0 = hg * NHC
        rows = np.concatenate(
            [
                w_qkv[h0 * DH : (h0 + NHC) * DH],
                w_qkv[D + h0 * DH : D + (h0 + NHC) * DH],
                w_qkv[2 * D + h0 * DH : 2 * D + (h0 + NHC) * DH],
            ],
            axis=0,
        )
        wqkvT[hg] = np.ascontiguousarray(rows.T).astype(BF)
        wfcTs[hg] = np.ascontiguousarray(w_fc.T[h0 * DH : (h0 + NHC) * DH]).astype(BF)
    xTs = [np.ascontiguousarray(x[b].T).astype(BF) for b in range(B)]
    in_maps = []
    for core in range(8):
        b, hg = core // 4, core % 4
        in_maps.append(
            {
                "xT": xTs[b],
                "wqkvT": wqkvT[hg],
                "wfcT": wfcTs[hg],
                "triu": triu_f,
                "wqk": wqk,
                "identb": identb,
                "identf": identf,
                "selhp": selhp,
            }
        )
    return in_maps


def _is_causal(mask):
    idx = np.arange(mask.shape[0])
    return mask.shape == (L, L) and bool(np.all(mask == (idx[None, :] <= idx[:, None])))


def _reference_numpy(x, mask, w_qkv, w_fc, q_norm_w, k_norm_w, subset_attention_size):
    # slow but general fallback (only used if mask is not causal)
    b, l, d = x.shape
    qkv = x @ w_qkv.T
    q, k, v = np.split(qkv, 3, axis=-1)

    def heads(t):
        return t.reshape(b, l, NH, DH).transpose(0, 2, 1, 3)

    def rms(t, w):
        return t * (1.0 / np.sqrt(np.mean(t * t, -1, keepdims=True) + EPS)) * w

    q, k, v = heads(q), heads(k), heads(v)
    q, k = rms(q, q_norm_w), rms(k, k_norm_w)

    def sdpa(q, k, v, m):
        s = np.einsum("bhqd,bhkd->bhqk", q, k) / np.sqrt(DH)
        s = np.where(m[None, None], s, -1e30)
        s = s - s.max(-1, keepdims=True)
        p = np.exp(s)
        p /= p.sum(-1, keepdims=True)
        return np.einsum("bhqk,bhkd->bhqd", p, v)

    S = int(subset_attention_size) if subset_attention_size is not None else None
    if S is not None and S < l:
        o = np.concatenate(
            [
                sdpa(q[:, :, :S], k[:, :, :S], v[:, :, :S], mask[:S, :S]),
                sdpa(q[:, :, S:], k, v, mask[S:, :]),
            ],
            axis=2,
        )
    else:
        o = sdpa(q, k, v, mask)
    o = o.transpose(0, 2, 1, 3).reshape(b, l, d)
    return (o @ w_fc.T).astype(np.float32)


def kernel(**inputs):
    x = np.asarray(inputs["x"], dtype=np.float32)
    mask = np.asarray(inputs["mask"])
    w_qkv = np.asarray(inputs["w_qkv"], dtype=np.float32)
    w_fc = np.asarray(inputs["w_fc"], dtype=np.float32)
    q_norm_w = np.asarray(inputs["q_norm_w"], dtype=np.float32)
    k_norm_w = np.asarray(inputs["k_norm_w"], dtype=np.float32)

    if not _is_causal(mask):
        return _reference_numpy(
            x, mask, w_qkv, w_fc, q_norm_w, k_norm_w,
            inputs.get("subset_attention_size"),
        )

    if "nc" not in _CACHE:
        _CACHE["nc"] = _build_nc()
    nc = _CACHE["nc"]

    in_maps = _make_in_maps(x, w_qkv, w_fc, q_norm_w, k_norm_w)
    res = run_bass_kernel_spmd(nc, in_maps, core_ids=list(range(8)))
    parts = [res.results[i]["outp"] for i in range(8)]
    out = np.empty((B, L, D), dtype=np.float32)
    for b in range(B):
        acc = np.zeros((L, D), dtype=np.float32)
        for hg in range(4):
            acc += parts[b * 4 + hg].astype(np.float32)
        out[b] = acc
    return out
